# revision 13
# baseline (speedup 1.0000x reference)
"""BiLSTM Trainium2 kernel (Bass/Tile), self-contained.

Problem: B=32, S=1024, D=512, H=512 bidirectional LSTM.
Sharding: 8 cores = 2 directions x 4 batch-groups of 8. All cores run ONE
program; the backward direction is realized by host-side time reversal of
its input slice (and un-reversal of its output).

Per-core device program (batch Bc=8, one direction):
  - Input projection xg = x @ Wx^T + (bx+bh), computed per 128-step chunk
    as a PE GEMM into SBUF (gates-major layout), fp16 inputs, fp32 accum.
  - Recurrence over S steps: g_t = xg_t + Wh @ h_{t-1} via 64 weight-
    stationary fp16 matmuls per step (N=8 batch columns), PSUM fp32;
    elementwise LSTM cell on DVE/ACT in fp32; h stored fp32 to an output
    ring, cast to fp16 for the next step's matmul rhs.

Layouts (p = partition 0..127, g = gate 0..3 (i,f,o,c), j = h-tile 0..3,
b = batch 0..7, s = step):
  xT_dev  [128, 4, S*8] fp16  : xT[p, k, s*8+b] = x[b, s, k*128+p]
  WxT_dev [128, 4, 2048] fp16 : WxT[p, k, c]    = Wx[c, k*128+p]
  WhT_dev [128, 4, 2048] fp16 : WhT[p, k, c]    = Wh[c, k*128+p]
  bias_dev[128, 16] fp32      : bias[p, g*4+j]  = (bx+bh)[g*512+j*128+p]
  hs_dev  [128, S*32] fp32    : hs[p, s*32+j*8+b] = h_s[j*128+p, b]
  hN_dev/cN_dev [128, 32] fp32: [p, j*8+b] = state[j*128+p, b]
"""

import numpy as np

import concourse.bass as bass
import concourse.mybir as mybir
import concourse.tile as tile
from concourse.bass import ds
from concourse.vector_clock import ScopedClock

B, S, D, H = 32, 1024, 512, 512
BC = 8          # batch per core
NCORES = 8
CH = 128        # steps per chunk
UNROLL = 16     # steps unrolled per For_i iteration
G4 = 4 * H      # 2048
KT = 4          # k tiles (D/128 == H/128)
MT = 16         # m tiles (2048/128)

FP16 = mybir.dt.float16
FP32 = mybir.dt.float32

_MAXW = 1


def _patched_drain_and_barrier(self, tick_clock, wait_clock):
    # This walrus build rejects Drain instructions with >1 sync waits
    # ("Too many sync wait commands"); split the exit drain.
    nc = self.nc
    drain_inst = nc.sync.drain()
    wait_clock.add_sem_waits(drain_inst.ins, ScopedClock({None: tick_clock.global_clock}))
    si = drain_inst.ins.sync_info
    waits = list(si.on_wait) if si is not None else []
    if len(waits) > _MAXW:
        SyncInfo = type(si)
        drain_inst.ins.sync_info = SyncInfo(on_wait=waits[:_MAXW], on_update=list(si.on_update))
        for i in range(_MAXW, len(waits), _MAXW):
            extra = nc.sync.drain()
            esi = extra.ins.sync_info
            ew = list(esi.on_wait) if esi is not None else []
            eu = list(esi.on_update) if esi is not None else []
            extra.ins.sync_info = SyncInfo(on_wait=ew + waits[i : i + _MAXW], on_update=eu)
    nc.all_engine_barrier()
    assert self.sems is not None
    popped = nc._tile_sem_poison_stack.pop()
    assert popped is self._sem_poison
    nc.clear_and_free_semaphores(list(self.sems.allocated().values()))
    nc.all_engine_barrier()


tile.TileContext._drain_and_barrier = _patched_drain_and_barrier


def _split_excess_waits(nc, maxw=_MAXW):
    """This walrus build caps sync waits per instruction; move overflow waits
    onto same-engine nops inserted just before the offending instruction."""
    cnt = 0
    for f in nc.m.functions:
        for bb in f.blocks:
            insts = bb.instructions
            i = 0
            while i < len(insts):
                ins = insts[i]
                si = ins.sync_info
                if si is not None and len(si.on_wait) > maxw:
                    waits = list(si.on_wait)
                    SyncInfo = type(si)
                    ins.sync_info = SyncInfo(
                        on_wait=waits[:maxw], on_update=list(si.on_update)
                    )
                    extra = waits[maxw:]
                    nops = []
                    for j in range(0, len(extra), maxw):
                        cnt += 1
                        nops.append(
                            mybir.InstNoOp(
                                name=f"WSPLIT-{cnt}",
                                engine=ins.engine,
                                ins=[],
                                outs=[],
                                sync_info=SyncInfo(
                                    on_wait=extra[j : j + maxw], on_update=[]
                                ),
                            )
                        )
                    for k, nop in enumerate(nops):
                        nc.register_instruction(nop)
                        insts.insert(i + k, nop)
                    i += len(nops)
                i += 1
    return cnt


def build_program(seq_len=S, ch=CH, unroll=UNROLL, repeats=1):
    """Build the per-core LSTM program. seq_len % ch == 0, ch % unroll == 0."""
    nchunks = seq_len // ch
    nc = bass.Bass()

    xT = nc.dram_tensor("xT", [128, KT, seq_len * BC], FP16, kind="ExternalInput")
    WxT = nc.dram_tensor("WxT", [128, KT, G4], FP16, kind="ExternalInput")
    WhT = nc.dram_tensor("WhT", [128, KT, G4], FP16, kind="ExternalInput")
    bias = nc.dram_tensor("bias", [128, 16], FP32, kind="ExternalInput")
    hs = nc.dram_tensor("hs", [128, seq_len * 32], FP32, kind="ExternalOutput")
    cN = nc.dram_tensor("cN", [128, 32], FP32, kind="ExternalOutput")

    with tile.TileContext(nc) as tc:
        with (
            tc.tile_pool(name="singles", bufs=1) as singles,
            tc.tile_pool(name="xtp", bufs=2) as xtp,
            tc.tile_pool(name="ringp", bufs=2) as ringp,
            tc.tile_pool(name="gemm_ps", bufs=2, space="PSUM") as gemm_ps,
            tc.tile_pool(name="rec_ps", bufs=2, space="PSUM") as rec_ps,
            tc.tile_pool(name="ew", bufs=2) as ew,
        ):
            WxT_sb = singles.tile([128, KT, G4], FP16)
            nc.sync.dma_start(out=WxT_sb, in_=WxT[:, :, :])
            WhT_sb = singles.tile([128, KT, G4], FP16)
            nc.sync.dma_start(out=WhT_sb, in_=WhT[:, :, :])
            bias_sb = singles.tile([128, 16], FP32)
            nc.sync.dma_start(out=bias_sb, in_=bias[:, :])

            xg = singles.tile([128, ch * 128], FP32)      # [p, s*128 + g*32 + j*8 + b]
            hT = singles.tile([128, 32], FP16)            # [p, j*8+b] = h[j*128+p, b]
            c_state = singles.tile([128, 32], FP32)
            ident = singles.tile([128, 128], FP32)
            from concourse.masks import make_identity

            make_identity(nc, ident)

            def lstm_pass():
                nc.vector.memset(hT, 0.0)
                nc.vector.memset(c_state, 0.0)

                for chunk in range(nchunks):
                    # ---- load x^T chunk ----
                    xT_sb = xtp.tile([128, KT, ch * BC], FP16, tag="xt")
                    nc.sync.dma_start(
                        out=xT_sb, in_=xT[:, :, chunk * ch * BC : (chunk + 1) * ch * BC]
                    )

                    # ---- chunk GEMM: xg = Wx @ x^T + bias (gates-major) ----
                    nhalf = (ch * BC) // 512 if (ch * BC) >= 512 else 1
                    hw = (ch * BC) // nhalf  # columns (s*8+b) per matmul group
                    xg_v = xg.rearrange("p (s q) -> p s q", q=128)
                    for m in range(MT):
                        g, j = divmod(m, KT)
                        for half in range(nhalf):
                            ps = gemm_ps.tile([128, hw], FP32, tag="gps")
                            for k in range(KT):
                                nc.tensor.matmul(
                                    ps,
                                    WxT_sb[:, k, m * 128 : (m + 1) * 128],
                                    xT_sb[:, k, half * hw : (half + 1) * hw],
                                    start=(k == 0),
                                    stop=(k == KT - 1),
                                )
                            sper = hw // BC
                            ps_v = ps.rearrange("p (s b) -> p s b", b=BC)
                            dst = xg_v[
                                :,
                                half * sper : (half + 1) * sper,
                                g * 32 + j * 8 : g * 32 + j * 8 + 8,
                            ]
                            nc.vector.tensor_scalar_add(
                                dst, ps_v, bias_sb[:, g * 4 + j : g * 4 + j + 1]
                            )

                    # ---- recurrence over ch steps ----
                    with tc.For_i(0, ch, unroll, hint_engines=(mybir.EngineType.PE,)) as iv:
                        hstage = ringp.tile([128, unroll * 32], FP32, tag="hstage")
                        xstage = ringp.tile([128, unroll * 128], FP32, tag="xstage")
                        nc.gpsimd.tensor_copy(xstage, xg[:, ds(iv * 128, unroll * 128)])
                        for u in range(unroll):
                            ps = rec_ps.tile([128, 128], FP32, tag="rps")
                            # seed psum with xg_t (identity matmul), then
                            # accumulate Wh @ h_{t-1} on top
                            nc.tensor.matmul(
                                ps,
                                ident,
                                xstage[:, u * 128 : (u + 1) * 128],
                                start=True,
                                stop=False,
                            )
                            for m in range(MT):
                                g, j = divmod(m, KT)
                                for k in range(KT):
                                    nc.tensor.matmul(
                                        ps[:, m * 8 : m * 8 + 8],
                                        WhT_sb[:, k, m * 128 : (m + 1) * 128],
                                        hT[:, k * 8 : k * 8 + 8],
                                        start=False,
                                        stop=(m == MT - 1 and k == KT - 1),
                                    )
                            sg = ew.tile([128, 96], FP32, tag="sg")
                            nc.scalar.activation(
                                sg, ps[:, 0:96], mybir.ActivationFunctionType.Sigmoid
                            )
                            tct = ew.tile([128, 32], FP32, tag="tct")
                            nc.scalar.activation(
                                tct, ps[:, 96:128], mybir.ActivationFunctionType.Tanh
                            )
                            t1 = ew.tile([128, 32], FP32, tag="t1")
                            nc.vector.tensor_mul(t1, sg[:, 32:64], c_state)
                            t2 = ew.tile([128, 32], FP32, tag="t2")
                            nc.vector.tensor_mul(t2, sg[:, 0:32], tct)
                            nc.vector.tensor_add(c_state, t1, t2)
                            tcs = ew.tile([128, 32], FP32, tag="tcs")
                            nc.scalar.activation(
                                tcs, c_state, mybir.ActivationFunctionType.Tanh
                            )
                            usl = slice(u * 32, (u + 1) * 32)
                            nc.vector.tensor_mul(hT, sg[:, 64:96], tcs)
                            nc.vector.tensor_mul(hstage[:, usl], sg[:, 64:96], tcs)
                        nc.sync.dma_start(
                            out=hs[:, ds(chunk * ch * 32 + iv * 32, unroll * 32)],
                            in_=hstage,
                        )
                    if chunk == nchunks - 1:
                        nc.sync.dma_start(out=cN[:, :], in_=c_state)

            if repeats == 1:
                lstm_pass()
            else:
                with tc.For_i(0, repeats, 1):
                    lstm_pass()

    _split_excess_waits(nc)
    return nc


# ---------------- host-side data prep ----------------

def _prep_core_inputs(x_slice, Wx, Wh, bx, bh, reverse):
    """x_slice [BC, S, D] fp32 -> per-core in_map."""
    xd = x_slice[:, ::-1] if reverse else x_slice
    seq_len = xd.shape[1]
    # xT[p, k, s*8+b] = xd[b, s, k*128+p]
    xT = np.ascontiguousarray(
        xd.transpose(2, 1, 0).reshape(KT, 128, seq_len * BC).transpose(1, 0, 2)
    ).astype(np.float16)
    WxT = np.ascontiguousarray(
        Wx.T.reshape(KT, 128, G4).transpose(1, 0, 2)
    ).astype(np.float16)
    WhT = np.ascontiguousarray(
        Wh.T.reshape(KT, 128, G4).transpose(1, 0, 2)
    ).astype(np.float16)
    bias = np.ascontiguousarray(
        (bx + bh).astype(np.float32).reshape(4, KT, 128).transpose(2, 0, 1).reshape(128, 16)
    )
    return {"xT": xT, "WxT": WxT, "WhT": WhT, "bias": bias}


def _decode_hs(hs_dev, seq_len):
    """hs_dev [128, S*32] -> h [BC, S, H]."""
    v = hs_dev.reshape(128, seq_len, KT, BC)   # [p, s, j, b]
    return v.transpose(3, 1, 2, 0).reshape(BC, seq_len, H)


def _decode_state(st_dev):
    """[128, 32] -> [BC, H]"""
    v = st_dev.reshape(128, KT, BC)            # [p, j, b]
    return v.transpose(2, 1, 0).reshape(BC, H)


_program_cache = {}


def _get_program(repeats=1):
    key = repeats
    if key not in _program_cache:
        _program_cache[key] = build_program(repeats=repeats)
    return _program_cache[key]


def run_device(inputs, repeats=1):
    """Run the SPMD program; returns list of 8 per-core result dicts."""
    from concourse.bass_utils import run_bass_kernel_spmd

    x = inputs["x"]
    in_maps = []
    for core in range(NCORES):
        d = core // 4
        bg = core % 4
        xs = x[bg * BC : (bg + 1) * BC]
        if d == 0:
            m = _prep_core_inputs(xs, inputs["Wx_f"], inputs["Wh_f"], inputs["bx_f"], inputs["bh_f"], False)
        else:
            m = _prep_core_inputs(xs, inputs["Wx_b"], inputs["Wh_b"], inputs["bx_b"], inputs["bh_b"], True)
        in_maps.append(m)
    nc = _get_program(repeats)
    res = run_bass_kernel_spmd(nc, in_maps, core_ids=list(range(NCORES)))
    return res.results


def assemble(results):
    outputs = np.empty((B, S, 2 * H), np.float32)
    final_cell = np.empty((B, 2 * H), np.float32)
    for core in range(NCORES):
        d = core // 4
        bg = core % 4
        bsl = slice(bg * BC, (bg + 1) * BC)
        hsd = _decode_hs(results[core]["hs"], S)
        if d == 1:
            hsd = hsd[:, ::-1]
        outputs[bsl, :, d * H : (d + 1) * H] = hsd
        final_cell[bsl, d * H : (d + 1) * H] = _decode_state(results[core]["cN"])
    final_hidden = np.concatenate([outputs[:, -1, :H], outputs[:, 0, H:]], axis=-1)
    return outputs, final_hidden, final_cell


def kernel(x, Wx_f, Wh_f, bx_f, bh_f, Wx_b, Wh_b, bx_b, bh_b):
    inputs = dict(x=np.asarray(x), Wx_f=np.asarray(Wx_f), Wh_f=np.asarray(Wh_f),
                  bx_f=np.asarray(bx_f), bh_f=np.asarray(bh_f),
                  Wx_b=np.asarray(Wx_b), Wh_b=np.asarray(Wh_b),
                  bx_b=np.asarray(bx_b), bh_b=np.asarray(bh_b))
    results = run_device(inputs, repeats=1)
    return assemble(results)


# revision 18
# speedup vs baseline: 1.0540x; 1.0540x over previous
"""BiLSTM Trainium2 kernel (Bass/Tile), self-contained.

Problem: B=32, S=1024, D=512, H=512 bidirectional LSTM.
Sharding: 8 cores = 2 directions x 4 batch-groups of 8. All cores run ONE
program; the backward direction is realized by host-side time reversal of
its input slice (and un-reversal of its output).

Per-core device program (batch Bc=8, one direction):
  - Input projection xg = x @ Wx^T + (bx+bh), computed per 128-step chunk
    as a PE GEMM into SBUF (gates-major layout), fp16 inputs, fp32 accum.
  - Recurrence over S steps: g_t = xg_t + Wh @ h_{t-1} via 64 weight-
    stationary fp16 matmuls per step (N=8 batch columns), PSUM fp32;
    elementwise LSTM cell on DVE/ACT in fp32; h stored fp32 to an output
    ring, cast to fp16 for the next step's matmul rhs.

Layouts (p = partition 0..127, g = gate 0..3 (i,f,o,c), j = h-tile 0..3,
b = batch 0..7, s = step):
  xT_dev  [128, 4, S*8] fp16  : xT[p, k, s*8+b] = x[b, s, k*128+p]
  WxT_dev [128, 4, 2048] fp16 : WxT[p, k, c]    = Wx[c, k*128+p]
  WhT_dev [128, 4, 2048] fp16 : WhT[p, k, c]    = Wh[c, k*128+p]
  bias_dev[128, 16] fp32      : bias[p, g*4+j]  = (bx+bh)[g*512+j*128+p]
  hs_dev  [128, S*32] fp32    : hs[p, s*32+j*8+b] = h_s[j*128+p, b]
  hN_dev/cN_dev [128, 32] fp32: [p, j*8+b] = state[j*128+p, b]
"""

import numpy as np

import concourse.bass as bass
import concourse.mybir as mybir
import concourse.tile as tile
from concourse.bass import ds
from concourse.vector_clock import ScopedClock

B, S, D, H = 32, 1024, 512, 512
BC = 8          # batch per core
NCORES = 8
CH = 128        # steps per chunk
UNROLL = 32     # steps unrolled per For_i iteration
G4 = 4 * H      # 2048
KT = 4          # k tiles (D/128 == H/128)
MT = 16         # m tiles (2048/128)

FP16 = mybir.dt.float16
FP32 = mybir.dt.float32

_MAXW = 1


def _patched_drain_and_barrier(self, tick_clock, wait_clock):
    # This walrus build rejects Drain instructions with >1 sync waits
    # ("Too many sync wait commands"); split the exit drain.
    nc = self.nc
    drain_inst = nc.sync.drain()
    wait_clock.add_sem_waits(drain_inst.ins, ScopedClock({None: tick_clock.global_clock}))
    si = drain_inst.ins.sync_info
    waits = list(si.on_wait) if si is not None else []
    if len(waits) > _MAXW:
        SyncInfo = type(si)
        drain_inst.ins.sync_info = SyncInfo(on_wait=waits[:_MAXW], on_update=list(si.on_update))
        for i in range(_MAXW, len(waits), _MAXW):
            extra = nc.sync.drain()
            esi = extra.ins.sync_info
            ew = list(esi.on_wait) if esi is not None else []
            eu = list(esi.on_update) if esi is not None else []
            extra.ins.sync_info = SyncInfo(on_wait=ew + waits[i : i + _MAXW], on_update=eu)
    nc.all_engine_barrier()
    assert self.sems is not None
    popped = nc._tile_sem_poison_stack.pop()
    assert popped is self._sem_poison
    nc.clear_and_free_semaphores(list(self.sems.allocated().values()))
    nc.all_engine_barrier()


tile.TileContext._drain_and_barrier = _patched_drain_and_barrier


def _split_excess_waits(nc, maxw=_MAXW):
    """This walrus build caps sync waits per instruction; move overflow waits
    onto same-engine nops inserted just before the offending instruction."""
    cnt = 0
    for f in nc.m.functions:
        for bb in f.blocks:
            insts = bb.instructions
            i = 0
            while i < len(insts):
                ins = insts[i]
                si = ins.sync_info
                if si is not None and len(si.on_wait) > maxw:
                    waits = list(si.on_wait)
                    SyncInfo = type(si)
                    ins.sync_info = SyncInfo(
                        on_wait=waits[:maxw], on_update=list(si.on_update)
                    )
                    extra = waits[maxw:]
                    nops = []
                    for j in range(0, len(extra), maxw):
                        cnt += 1
                        nops.append(
                            mybir.InstNoOp(
                                name=f"WSPLIT-{cnt}",
                                engine=ins.engine,
                                ins=[],
                                outs=[],
                                sync_info=SyncInfo(
                                    on_wait=extra[j : j + maxw], on_update=[]
                                ),
                            )
                        )
                    for k, nop in enumerate(nops):
                        nc.register_instruction(nop)
                        insts.insert(i + k, nop)
                    i += len(nops)
                i += 1
    return cnt


def build_program(seq_len=S, ch=CH, unroll=UNROLL, repeats=1):
    """Build the per-core LSTM program. seq_len % ch == 0, ch % unroll == 0."""
    nchunks = seq_len // ch
    nc = bass.Bass()

    xT = nc.dram_tensor("xT", [128, KT, seq_len * BC], FP16, kind="ExternalInput")
    WxT = nc.dram_tensor("WxT", [128, KT, G4], FP16, kind="ExternalInput")
    WhT = nc.dram_tensor("WhT", [128, KT, G4], FP16, kind="ExternalInput")
    bias = nc.dram_tensor("bias", [128, 16], FP32, kind="ExternalInput")
    hs = nc.dram_tensor("hs", [128, seq_len * 32], FP32, kind="ExternalOutput")
    cN = nc.dram_tensor("cN", [128, 32], FP32, kind="ExternalOutput")

    with tile.TileContext(nc) as tc:
        with (
            tc.tile_pool(name="singles", bufs=1) as singles,
            tc.tile_pool(name="xtp", bufs=2) as xtp,
            tc.tile_pool(name="ringp", bufs=2) as ringp,
            tc.tile_pool(name="gemm_ps", bufs=2, space="PSUM") as gemm_ps,
            tc.tile_pool(name="rec_ps", bufs=2, space="PSUM") as rec_ps,
            tc.tile_pool(name="ew", bufs=2) as ew,
        ):
            WxT_sb = singles.tile([128, KT, G4], FP16)
            nc.sync.dma_start(out=WxT_sb, in_=WxT[:, :, :])
            WhT_sb = singles.tile([128, KT, G4], FP16)
            nc.sync.dma_start(out=WhT_sb, in_=WhT[:, :, :])
            bias_sb = singles.tile([128, 16], FP32)
            nc.sync.dma_start(out=bias_sb, in_=bias[:, :])

            xg = singles.tile([128, ch * 128], FP32)      # [p, s*128 + g*32 + j*8 + b]
            # h^T for the next step's matmul rhs, split per k-tile so the
            # next step's first matmuls can start as soon as slice 0 is ready
            hTk = [
                singles.tile([128, 8], FP16, name=f"hT{k}", tag=f"hT{k}")
                for k in range(KT)
            ]
            c_state = singles.tile([128, 32], FP32)

            def lstm_pass():
                for k in range(KT):
                    nc.vector.memset(hTk[k], 0.0)
                nc.vector.memset(c_state, 0.0)

                for chunk in range(nchunks):
                    # ---- load x^T chunk ----
                    xT_sb = xtp.tile([128, KT, ch * BC], FP16, tag="xt")
                    nc.sync.dma_start(
                        out=xT_sb, in_=xT[:, :, chunk * ch * BC : (chunk + 1) * ch * BC]
                    )

                    # ---- chunk GEMM: xg = Wx @ x^T + bias (gates-major) ----
                    nhalf = (ch * BC) // 512 if (ch * BC) >= 512 else 1
                    hw = (ch * BC) // nhalf  # columns (s*8+b) per matmul group
                    xg_v = xg.rearrange("p (s q) -> p s q", q=128)
                    for m in range(MT):
                        g, j = divmod(m, KT)
                        for half in range(nhalf):
                            ps = gemm_ps.tile([128, hw], FP32, tag="gps")
                            for k in range(KT):
                                nc.tensor.matmul(
                                    ps,
                                    WxT_sb[:, k, m * 128 : (m + 1) * 128],
                                    xT_sb[:, k, half * hw : (half + 1) * hw],
                                    start=(k == 0),
                                    stop=(k == KT - 1),
                                )
                            sper = hw // BC
                            ps_v = ps.rearrange("p (s b) -> p s b", b=BC)
                            dst = xg_v[
                                :,
                                half * sper : (half + 1) * sper,
                                g * 32 + j * 8 : g * 32 + j * 8 + 8,
                            ]
                            nc.vector.tensor_scalar_add(
                                dst, ps_v, bias_sb[:, g * 4 + j : g * 4 + j + 1]
                            )

                    # ---- recurrence over ch steps ----
                    with tc.For_i(0, ch, unroll, hint_engines=(mybir.EngineType.PE, mybir.EngineType.DVE)) as iv:
                        hstage = ringp.tile([128, unroll * 32], FP32, tag="hstage")
                        xstage = ringp.tile([128, unroll * 128], FP32, tag="xstage")
                        # DVE on purpose: xg's writers (GEMM bias-adds) and
                        # xstage's readers (gate adds) are DVE, so program
                        # order covers the dynamic-offset hazard.
                        nc.vector.tensor_copy(xstage, xg[:, ds(iv * 128, unroll * 128)])
                        for u in range(unroll):
                            # per-gate psum tiles; matmul group order ct,i,f,o
                            # so each gate's elementwise work overlaps the
                            # remaining matmuls and only the o-gate tail is
                            # on the critical path into the next step.
                            pss = {}
                            acts = {}
                            for g, fn in (
                                (3, mybir.ActivationFunctionType.Tanh),   # ct
                                (0, mybir.ActivationFunctionType.Sigmoid),  # i
                                (1, mybir.ActivationFunctionType.Sigmoid),  # f
                                (2, mybir.ActivationFunctionType.Sigmoid),  # o
                            ):
                                ps = rec_ps.tile(
                                    [128, 32], FP32, name=f"ps{g}", tag=f"ps{g}", bufs=1
                                )
                                pss[g] = ps
                                for j in range(KT):
                                    m = g * KT + j
                                    for k in range(KT):
                                        nc.tensor.matmul(
                                            ps[:, j * 8 : j * 8 + 8],
                                            WhT_sb[:, k, m * 128 : (m + 1) * 128],
                                            hTk[k],
                                            start=(k == 0),
                                            stop=(k == KT - 1),
                                        )
                                gb = ew.tile([128, 32], FP32, name=f"gb{g}", tag=f"gb{g}")
                                nc.vector.tensor_add(
                                    gb,
                                    ps,
                                    xstage[:, u * 128 + g * 32 : u * 128 + g * 32 + 32],
                                )
                                av = ew.tile([128, 32], FP32, name=f"av{g}", tag=f"av{g}")
                                nc.scalar.activation(av, gb, fn)
                                acts[g] = av
                            t1 = ew.tile([128, 32], FP32, tag="t1")
                            nc.vector.tensor_mul(t1, acts[1], c_state)
                            t2 = ew.tile([128, 32], FP32, tag="t2")
                            nc.vector.tensor_mul(t2, acts[0], acts[3])
                            nc.vector.tensor_add(c_state, t1, t2)
                            tcs = ew.tile([128, 32], FP32, tag="tcs")
                            nc.scalar.activation(
                                tcs, c_state, mybir.ActivationFunctionType.Tanh
                            )
                            for k in range(KT):
                                nc.vector.tensor_mul(
                                    hTk[k],
                                    acts[2][:, k * 8 : k * 8 + 8],
                                    tcs[:, k * 8 : k * 8 + 8],
                                )
                            usl = slice(u * 32, (u + 1) * 32)
                            nc.vector.tensor_mul(hstage[:, usl], acts[2], tcs)
                        nc.sync.dma_start(
                            out=hs[:, ds(chunk * ch * 32 + iv * 32, unroll * 32)],
                            in_=hstage,
                        )
                    if chunk == nchunks - 1:
                        nc.sync.dma_start(out=cN[:, :], in_=c_state)

            if repeats == 1:
                lstm_pass()
            else:
                with tc.For_i(0, repeats, 1):
                    lstm_pass()

    _split_excess_waits(nc)
    return nc


# ---------------- host-side data prep ----------------

def _prep_core_inputs(x_slice, Wx, Wh, bx, bh, reverse):
    """x_slice [BC, S, D] fp32 -> per-core in_map."""
    xd = x_slice[:, ::-1] if reverse else x_slice
    seq_len = xd.shape[1]
    # xT[p, k, s*8+b] = xd[b, s, k*128+p]
    xT = np.ascontiguousarray(
        xd.transpose(2, 1, 0).reshape(KT, 128, seq_len * BC).transpose(1, 0, 2)
    ).astype(np.float16)
    WxT = np.ascontiguousarray(
        Wx.T.reshape(KT, 128, G4).transpose(1, 0, 2)
    ).astype(np.float16)
    WhT = np.ascontiguousarray(
        Wh.T.reshape(KT, 128, G4).transpose(1, 0, 2)
    ).astype(np.float16)
    bias = np.ascontiguousarray(
        (bx + bh).astype(np.float32).reshape(4, KT, 128).transpose(2, 0, 1).reshape(128, 16)
    )
    return {"xT": xT, "WxT": WxT, "WhT": WhT, "bias": bias}


def _decode_hs(hs_dev, seq_len):
    """hs_dev [128, S*32] -> h [BC, S, H]."""
    v = hs_dev.reshape(128, seq_len, KT, BC)   # [p, s, j, b]
    return v.transpose(3, 1, 2, 0).reshape(BC, seq_len, H)


def _decode_state(st_dev):
    """[128, 32] -> [BC, H]"""
    v = st_dev.reshape(128, KT, BC)            # [p, j, b]
    return v.transpose(2, 1, 0).reshape(BC, H)


_program_cache = {}


def _get_program(repeats=1):
    key = repeats
    if key not in _program_cache:
        _program_cache[key] = build_program(repeats=repeats)
    return _program_cache[key]


def run_device(inputs, repeats=1):
    """Run the SPMD program; returns list of 8 per-core result dicts."""
    from concourse.bass_utils import run_bass_kernel_spmd

    x = inputs["x"]
    in_maps = []
    for core in range(NCORES):
        d = core // 4
        bg = core % 4
        xs = x[bg * BC : (bg + 1) * BC]
        if d == 0:
            m = _prep_core_inputs(xs, inputs["Wx_f"], inputs["Wh_f"], inputs["bx_f"], inputs["bh_f"], False)
        else:
            m = _prep_core_inputs(xs, inputs["Wx_b"], inputs["Wh_b"], inputs["bx_b"], inputs["bh_b"], True)
        in_maps.append(m)
    nc = _get_program(repeats)
    res = run_bass_kernel_spmd(nc, in_maps, core_ids=list(range(NCORES)))
    return res.results


def assemble(results):
    outputs = np.empty((B, S, 2 * H), np.float32)
    final_cell = np.empty((B, 2 * H), np.float32)
    for core in range(NCORES):
        d = core // 4
        bg = core % 4
        bsl = slice(bg * BC, (bg + 1) * BC)
        hsd = _decode_hs(results[core]["hs"], S)
        if d == 1:
            hsd = hsd[:, ::-1]
        outputs[bsl, :, d * H : (d + 1) * H] = hsd
        final_cell[bsl, d * H : (d + 1) * H] = _decode_state(results[core]["cN"])
    final_hidden = np.concatenate([outputs[:, -1, :H], outputs[:, 0, H:]], axis=-1)
    return outputs, final_hidden, final_cell


def kernel(x, Wx_f, Wh_f, bx_f, bh_f, Wx_b, Wh_b, bx_b, bh_b):
    inputs = dict(x=np.asarray(x), Wx_f=np.asarray(Wx_f), Wh_f=np.asarray(Wh_f),
                  bx_f=np.asarray(bx_f), bh_f=np.asarray(bh_f),
                  Wx_b=np.asarray(Wx_b), Wh_b=np.asarray(Wh_b),
                  bx_b=np.asarray(bx_b), bh_b=np.asarray(bh_b))
    results = run_device(inputs, repeats=1)
    return assemble(results)
